# revision 1
# baseline (speedup 1.0000x reference)
"""Trainium2 Bass kernel for nn_LocalGeometryEncoding (v4).

Gather mechanism is SWDGE indirect DMA (the only performant indirect path
on this HW: ~1.2us per 128-row launch, engine-serial).  Structure:
For_i over 25 super-tiles of 512 points; per iteration 128 per-k gather
launches round-robin over 4 SWDGE queues, then N=512 matmul streams.
staggered_reset avoids the full all-engine stall of plain For_i.

Host prep identical to baseline (table channel-last [262144, 8] f32 with
row 0 zeroed; idx int32 [128, NT*K] tile-major; weights packed for PE).
"""

import numpy as np

import concourse.bacc as bacc
import concourse.bass as bass
import concourse.mybir as mybir
import concourse.tile as tile
from concourse.bass_utils import run_bass_kernel_spmd
from concourse.masks import make_identity

NGRID = 262144
P = 128
K = 32
C = 8
DIN = 256  # C*K
BASE = 512
NCORES = 8
NPC = 12500  # points per core
ST = 512  # points per super-tile
NST = 25  # super-tiles per core
NPAD = NST * ST  # 12800
NT = NPAD // P  # 100 partition-tiles
NQ = 4  # SWDGE queues

F32 = mybir.dt.float32
I32 = mybir.dt.int32

_CACHE = {}


def _build(reps: int = 1, staggered: bool = True):
    nc = bacc.Bacc(
        "TRN2", target_bir_lowering=False, debug=False, num_swdge_queues=NQ
    )
    table = nc.dram_tensor("table", [NGRID, C], F32, kind="ExternalInput")
    idxs = nc.dram_tensor("idxs", [P, NT * K], I32, kind="ExternalInput")
    w1 = nc.dram_tensor("w1", [P, 2 * BASE], F32, kind="ExternalInput")
    w2 = nc.dram_tensor("w2", [P, 4 * K], F32, kind="ExternalInput")
    b1 = nc.dram_tensor("b1", [P, 4], F32, kind="ExternalInput")
    b2 = nc.dram_tensor("b2", [K, 1], F32, kind="ExternalInput")
    outT = nc.dram_tensor("outT", [K, NPAD], F32, kind="ExternalOutput")

    Relu = mybir.ActivationFunctionType.Relu
    Ident = mybir.ActivationFunctionType.Identity

    n_iter = NST * reps
    with tile.TileContext(nc) as tc:
        with (
            tc.tile_pool(name="const", bufs=1) as cp,
            tc.tile_pool(name="work", bufs=2) as wp,
            tc.tile_pool(name="psx", bufs=1, space="PSUM") as psx,
            tc.tile_pool(name="psh", bufs=3, space="PSUM") as psh,
            tc.tile_pool(name="pso", bufs=2, space="PSUM") as pso,
        ):
            idx_sb = cp.tile([P, NT * K], I32)
            nc.sync.dma_start(idx_sb[:], idxs[:, :])
            w1_sb = cp.tile([P, 2 * BASE], F32)
            nc.sync.dma_start(w1_sb[:], w1[:, :])
            w2_sb = cp.tile([P, 4 * K], F32)
            nc.sync.dma_start(w2_sb[:], w2[:, :])
            b1_sb = cp.tile([P, 4], F32)
            nc.sync.dma_start(b1_sb[:], b1[:, :])
            b2_sb = cp.tile([K, 1], F32)
            nc.sync.dma_start(b2_sb[:], b2[:, :])
            ident = cp.tile([P, P], F32)
            make_identity(nc, ident[:])

            def supertile_body(t):
                # t is a For_i register; in timing mode (reps>1) the source
                # and dest slices are pinned so t may exceed NST.
                src_col = bass.ds(t * (4 * K), 4 * K) if reps == 1 else bass.ds(0, 4 * K)
                out_col = bass.ds(t * ST, ST) if reps == 1 else bass.ds(0, ST)
                stage = wp.tile([P, 4 * K], I32, tag="stage", bufs=2, name="stage")
                nc.sync.dma_start(stage[:], idx_sb[:, src_col])
                x4 = wp.tile([P, 4 * DIN], F32, tag="x4", bufs=2, name="x4")
                for j in range(4):
                    for k in range(K):
                        g = nc.gpsimd.indirect_dma_start(
                            out=x4[:, j * DIN + k * C : j * DIN + (k + 1) * C],
                            out_offset=None,
                            in_=table[:, :],
                            in_offset=bass.IndirectOffsetOnAxis(
                                ap=stage[:, j * K + k : j * K + k + 1], axis=0
                            ),
                        )
                        q = (j * K + k) % NQ
                        if q:
                            g.ins.queue = f"qPoolDynamic{q}"
                # transpose to feature-major: xT[h] = [128 feats, 512 pts]
                xT = []
                for h in range(2):
                    xp = psx.tile([P, ST], F32, tag=f"xp{h}", name=f"xp{h}")
                    for j in range(4):
                        nc.tensor.transpose(
                            xp[:, j * P : (j + 1) * P],
                            x4[:, j * DIN + h * P : j * DIN + (h + 1) * P],
                            ident[:],
                        )
                    xs = wp.tile([P, ST], F32, tag=f"xT{h}", bufs=2, name=f"xT{h}")
                    nc.vector.tensor_copy(xs[:], xp[:])
                    xT.append(xs)
                # L1
                hT = []
                for fo in range(4):
                    ph = psh.tile([P, ST], F32, tag="ph", name=f"ph{fo}")
                    nc.tensor.matmul(
                        ph[:],
                        lhsT=w1_sb[:, fo * P : (fo + 1) * P],
                        rhs=xT[0][:],
                        start=True,
                        stop=False,
                    )
                    nc.tensor.matmul(
                        ph[:],
                        lhsT=w1_sb[:, BASE + fo * P : BASE + (fo + 1) * P],
                        rhs=xT[1][:],
                        start=False,
                        stop=True,
                    )
                    hs = wp.tile([P, ST], F32, tag=f"hT{fo}", bufs=2, name=f"hT{fo}")
                    nc.scalar.activation(
                        hs[:], ph[:], Relu, bias=b1_sb[:, fo : fo + 1]
                    )
                    hT.append(hs)
                # L2
                po = pso.tile([K, ST], F32, tag="po", name="po")
                for fo in range(4):
                    nc.tensor.matmul(
                        po[:],
                        lhsT=w2_sb[:, fo * K : (fo + 1) * K],
                        rhs=hT[fo][:],
                        start=(fo == 0),
                        stop=(fo == 3),
                    )
                ot = wp.tile([K, ST], F32, tag="ot", bufs=2, name="ot")
                nc.scalar.activation(ot[:], po[:], Ident, bias=b2_sb[:, 0:1])
                nc.sync.dma_start(outT[:, out_col], ot[:])

            with tc.For_i(0, n_iter, 1, staggered_reset=staggered) as i:
                supertile_body(i)

    nc.compile()
    return nc


def _prep(encoding_g, mapping, W1, b1, W2, b2):
    """Host-side input prep: sharding + layout (no gather/compute)."""
    table = np.ascontiguousarray(
        np.asarray(encoding_g, dtype=np.float32).reshape(C, NGRID).T
    )
    table[0, :] = 0.0  # index 0 marks invalid -> gathers zero

    W1 = np.asarray(W1, dtype=np.float32)
    r = np.arange(DIN)
    W1p = W1[(r % C) * K + (r // C), :]  # row k*8+c <- W1 row c*32+k
    w1d = np.ascontiguousarray(
        np.concatenate([W1p[:P, :], W1p[P:, :]], axis=1)
    )  # [128, 1024]
    W2 = np.asarray(W2, dtype=np.float32)
    w2d = np.ascontiguousarray(
        W2.reshape(4, P, K).transpose(1, 0, 2).reshape(P, 4 * K)
    )
    b1d = np.ascontiguousarray(
        np.asarray(b1, dtype=np.float32).reshape(4, P).T
    )
    b2d = np.ascontiguousarray(np.asarray(b2, dtype=np.float32).reshape(K, 1))

    m = np.asarray(mapping).reshape(NCORES * NPC, K)
    in_maps = []
    for core in range(NCORES):
        mc = m[core * NPC : (core + 1) * NPC]
        mcp = np.zeros((NPAD, K), dtype=np.int32)
        mcp[:NPC] = mc.astype(np.int32)
        idxs = np.ascontiguousarray(
            mcp.reshape(NT, P, K).transpose(1, 0, 2).reshape(P, NT * K)
        )
        in_maps.append(
            {"table": table, "idxs": idxs, "w1": w1d, "w2": w2d, "b1": b1d, "b2": b2d}
        )
    return in_maps


def kernel(encoding_g, mapping, W1, b1, W2, b2):
    if "nc" not in _CACHE:
        _CACHE["nc"] = _build()
    nc = _CACHE["nc"]
    in_maps = _prep(encoding_g, mapping, W1, b1, W2, b2)
    res = run_bass_kernel_spmd(nc, in_maps, core_ids=list(range(NCORES)))
    outs = [r["outT"].T[:NPC] for r in res.results]
    out = np.concatenate(outs, axis=0).reshape(1, NCORES * NPC, K)
    return np.ascontiguousarray(out.astype(np.float32))



# revision 4
# speedup vs baseline: 1.2757x; 1.2757x over previous
"""Trainium2 Bass kernel for nn_LocalGeometryEncoding (v4u5).

Gather mechanism is SWDGE indirect DMA (the only performant indirect path
on this HW: ~1us per 128-row launch, engine-serial; the ucode consumes one
cross-partition vector of 128 indices per launch, so larger offset APs
don't batch, and the batched InstDMAGatherAnt path measures ~200us/launch
here).  Structure: For_i with FIVE 512-point super-tiles per iteration
(5x unroll cuts the staggered-reset count 5x, ~10%); per super-tile 128
per-k gather launches round-robin over 4 SWDGE queues, then N=512 matmul
streams.

Host prep identical to baseline (table channel-last [262144, 8] f32 with
row 0 zeroed; idx int32 [128, NT*K] tile-major; weights packed for PE).
"""

import numpy as np

import concourse.bacc as bacc
import concourse.bass as bass
import concourse.mybir as mybir
import concourse.tile as tile
from concourse.bass_utils import run_bass_kernel_spmd
from concourse.masks import make_identity

NGRID = 262144
P = 128
K = 32
C = 8
DIN = 256  # C*K
BASE = 512
NCORES = 8
NPC = 12500  # points per core
ST = 512  # points per super-tile
NST = 25  # super-tiles per core
NPAD = NST * ST  # 12800
NT = NPAD // P  # 100 partition-tiles
NQ = 4  # SWDGE queues

F32 = mybir.dt.float32
I32 = mybir.dt.int32

_CACHE = {}


def _build(reps: int = 1, staggered: bool = True):
    nc = bacc.Bacc(
        "TRN2", target_bir_lowering=False, debug=False, num_swdge_queues=NQ
    )
    table = nc.dram_tensor("table", [NGRID, C], F32, kind="ExternalInput")
    idxs = nc.dram_tensor("idxs", [P, NT * K], I32, kind="ExternalInput")
    w1 = nc.dram_tensor("w1", [P, 2 * BASE], F32, kind="ExternalInput")
    w2 = nc.dram_tensor("w2", [P, 4 * K], F32, kind="ExternalInput")
    b1 = nc.dram_tensor("b1", [P, 4], F32, kind="ExternalInput")
    b2 = nc.dram_tensor("b2", [K, 1], F32, kind="ExternalInput")
    outT = nc.dram_tensor("outT", [K, NPAD], F32, kind="ExternalOutput")

    Relu = mybir.ActivationFunctionType.Relu
    Ident = mybir.ActivationFunctionType.Identity

    n_iter = (NST // 5) * reps  # 5 super-tiles per hw-loop iteration
    with tile.TileContext(nc) as tc:
        with (
            tc.tile_pool(name="const", bufs=1) as cp,
            tc.tile_pool(name="work", bufs=2) as wp,
            tc.tile_pool(name="psx", bufs=1, space="PSUM") as psx,
            tc.tile_pool(name="psh", bufs=3, space="PSUM") as psh,
            tc.tile_pool(name="pso", bufs=2, space="PSUM") as pso,
        ):
            idx_sb = cp.tile([P, NT * K], I32)
            nc.sync.dma_start(idx_sb[:], idxs[:, :])
            w1_sb = cp.tile([P, 2 * BASE], F32)
            nc.sync.dma_start(w1_sb[:], w1[:, :])
            w2_sb = cp.tile([P, 4 * K], F32)
            nc.sync.dma_start(w2_sb[:], w2[:, :])
            b1_sb = cp.tile([P, 4], F32)
            nc.sync.dma_start(b1_sb[:], b1[:, :])
            b2_sb = cp.tile([K, 1], F32)
            nc.sync.dma_start(b2_sb[:], b2[:, :])
            ident = cp.tile([P, P], F32)
            make_identity(nc, ident[:])

            def supertile_body(t):
                # t is a For_i register; in timing mode (reps>1) the source
                # and dest slices are pinned so t may exceed NST.
                src_col = bass.ds(t * (4 * K), 4 * K) if reps == 1 else bass.ds(0, 4 * K)
                out_col = bass.ds(t * ST, ST) if reps == 1 else bass.ds(0, ST)
                stage = wp.tile([P, 4 * K], I32, tag="stage", bufs=2, name="stage")
                nc.sync.dma_start(stage[:], idx_sb[:, src_col])
                x4 = wp.tile([P, 4 * DIN], F32, tag="x4", bufs=2, name="x4")
                for j in range(4):
                    for k in range(K):
                        g = nc.gpsimd.indirect_dma_start(
                            out=x4[:, j * DIN + k * C : j * DIN + (k + 1) * C],
                            out_offset=None,
                            in_=table[:, :],
                            in_offset=bass.IndirectOffsetOnAxis(
                                ap=stage[:, j * K + k : j * K + k + 1], axis=0
                            ),
                        )
                        q = (j * K + k) % NQ
                        if q:
                            g.ins.queue = f"qPoolDynamic{q}"
                # transpose to feature-major: xT[h] = [128 feats, 512 pts]
                xT = []
                for h in range(2):
                    xp = psx.tile([P, ST], F32, tag=f"xp{h}", name=f"xp{h}")
                    for j in range(4):
                        nc.tensor.transpose(
                            xp[:, j * P : (j + 1) * P],
                            x4[:, j * DIN + h * P : j * DIN + (h + 1) * P],
                            ident[:],
                        )
                    xs = wp.tile([P, ST], F32, tag=f"xT{h}", bufs=2, name=f"xT{h}")
                    nc.vector.tensor_copy(xs[:], xp[:])
                    xT.append(xs)
                # L1
                hT = []
                for fo in range(4):
                    ph = psh.tile([P, ST], F32, tag="ph", name=f"ph{fo}")
                    nc.tensor.matmul(
                        ph[:],
                        lhsT=w1_sb[:, fo * P : (fo + 1) * P],
                        rhs=xT[0][:],
                        start=True,
                        stop=False,
                    )
                    nc.tensor.matmul(
                        ph[:],
                        lhsT=w1_sb[:, BASE + fo * P : BASE + (fo + 1) * P],
                        rhs=xT[1][:],
                        start=False,
                        stop=True,
                    )
                    hs = wp.tile([P, ST], F32, tag=f"hT{fo}", bufs=2, name=f"hT{fo}")
                    nc.scalar.activation(
                        hs[:], ph[:], Relu, bias=b1_sb[:, fo : fo + 1]
                    )
                    hT.append(hs)
                # L2
                po = pso.tile([K, ST], F32, tag="po", name="po")
                for fo in range(4):
                    nc.tensor.matmul(
                        po[:],
                        lhsT=w2_sb[:, fo * K : (fo + 1) * K],
                        rhs=hT[fo][:],
                        start=(fo == 0),
                        stop=(fo == 3),
                    )
                ot = wp.tile([K, ST], F32, tag="ot", bufs=2, name="ot")
                nc.scalar.activation(ot[:], po[:], Ident, bias=b2_sb[:, 0:1])
                nc.sync.dma_start(outT[:, out_col], ot[:])

            with tc.For_i(0, n_iter, 1, staggered_reset=staggered) as i:
                for u in range(5):
                    supertile_body(i * 5 + u)

    nc.compile()
    return nc


def _prep(encoding_g, mapping, W1, b1, W2, b2):
    """Host-side input prep: sharding + layout (no gather/compute)."""
    table = np.ascontiguousarray(
        np.asarray(encoding_g, dtype=np.float32).reshape(C, NGRID).T
    )
    table[0, :] = 0.0  # index 0 marks invalid -> gathers zero

    W1 = np.asarray(W1, dtype=np.float32)
    r = np.arange(DIN)
    W1p = W1[(r % C) * K + (r // C), :]  # row k*8+c <- W1 row c*32+k
    w1d = np.ascontiguousarray(
        np.concatenate([W1p[:P, :], W1p[P:, :]], axis=1)
    )  # [128, 1024]
    W2 = np.asarray(W2, dtype=np.float32)
    w2d = np.ascontiguousarray(
        W2.reshape(4, P, K).transpose(1, 0, 2).reshape(P, 4 * K)
    )
    b1d = np.ascontiguousarray(
        np.asarray(b1, dtype=np.float32).reshape(4, P).T
    )
    b2d = np.ascontiguousarray(np.asarray(b2, dtype=np.float32).reshape(K, 1))

    m = np.asarray(mapping).reshape(NCORES * NPC, K)
    in_maps = []
    for core in range(NCORES):
        mc = m[core * NPC : (core + 1) * NPC]
        mcp = np.zeros((NPAD, K), dtype=np.int32)
        mcp[:NPC] = mc.astype(np.int32)
        idxs = np.ascontiguousarray(
            mcp.reshape(NT, P, K).transpose(1, 0, 2).reshape(P, NT * K)
        )
        in_maps.append(
            {"table": table, "idxs": idxs, "w1": w1d, "w2": w2d, "b1": b1d, "b2": b2d}
        )
    return in_maps


def kernel(encoding_g, mapping, W1, b1, W2, b2):
    if "nc" not in _CACHE:
        _CACHE["nc"] = _build()
    nc = _CACHE["nc"]
    in_maps = _prep(encoding_g, mapping, W1, b1, W2, b2)
    res = run_bass_kernel_spmd(nc, in_maps, core_ids=list(range(NCORES)))
    outs = [r["outT"].T[:NPC] for r in res.results]
    out = np.concatenate(outs, axis=0).reshape(1, NCORES * NPC, K)
    return np.ascontiguousarray(out.astype(np.float32))

